# revision 1
# baseline (speedup 1.0000x reference)
"""Trainium2 Bass kernel: batched FFT along axis 1 of x[64, 4096, 128] (fp32),
returning (real, imag) parts.  8-core data-parallel over the batch axis.

Algorithm (per batch slice [4096, 128]): 4-step Cooley-Tukey with
N = N1*N2 = 128*32, n = 32*n1 + n2, k = 128*k2 + k1:

    X[128*k2 + k1] = sum_n2 T[k1,n2] * W32[n2,k2] * (sum_n1 W128[n1,k1] * x[32*n1+n2])

Phase 1 (per batch):
  - load x as [p=n1, f=n2*128+m]
  - stage-1 DFT-128 over n1 on the PE, rows k1 = 0..64 only (x is real, so
    A[128-k1] = conj(A[k1]); column 64 of the DFT matrix is the Nyquist row)
  - twiddle B = A*exp(-2i*pi*k1*n2/4096) on the DVE (PSUM -> SBUF)
  - write B[65, 4096] to an internal DRAM bounce buffer in n2-major order.
The DRAM round trip performs the k1<->n2 transpose: SBUF<->SBUF DMA cannot
cross partitions, and matmuls cannot target PSUM partitions >= 64 (PE
tiling is broken for 4-byte dtypes), so a [k1 x n2]-packed on-chip
transpose is not expressible.  DRAM APs are unrestricted.

Phase 2 (per batch):
  - load Bd[p=32g+n2, f=jm*128+m] = B[q=32g+jm, n2, m] for g in {0,1}
    (rows k1=0..63), 4KiB-contiguous reads
  - stage-2 DFT-32 over n2 with wide [64,128] stationaries: one matmul
    produces both the direct outputs (k1 = 32g+jm) and the conjugate
    outputs (k1' = 128-q) from the same moving pass, conjugation signs
    folded into the stationary constants.  The Nyquist row feeds a tiny
    separate matmul (its conjugate fixed point needs no sign fixup).
  - ACT evicts PSUM -> SBUF, DMA out in natural k row order.
"""

import numpy as np
from contextlib import ExitStack

import concourse.bacc as bacc
import concourse.bass as bass
import concourse.mybir as mybir
import concourse.tile as tile
from concourse.bass_utils import run_bass_kernel_spmd

N = 4096
N1, N2 = 128, 32
M = 128
B_FULL = 64
NCORES = 8
BPER = B_FULL // NCORES  # 8 batches per core

FP32 = mybir.dt.float32
FP32R = mybir.dt.float32r  # full-rate fp32 matmul streaming format

QROWS = 65           # stored B rows q = 0..64 (Hermitian half + Nyquist)
BD_ROW = QROWS * M   # DRAM bounce stride per n2, in elements


# ---------------------------------------------------------------- constants
def make_consts():
    n1 = np.arange(N1)
    k1 = np.arange(QROWS)
    ang1 = 2 * np.pi * np.outer(n1, k1) / N1
    g_mat = np.cos(ang1).astype(np.float32)              # [128, 65]
    h_mat = (-np.sin(ang1)).astype(np.float32)           # [128, 65]

    n2 = np.arange(N2)
    ang_t = 2 * np.pi * np.outer(k1, n2) / N
    t_re = np.cos(ang_t).astype(np.float32)              # [65, 32] m-bcast
    t_im = (-np.sin(ang_t)).astype(np.float32)

    # stage 2 blocks
    k2v = np.arange(N2)
    a2 = 2 * np.pi * np.outer(n2, k2v) / N2
    a2u = 2 * np.pi * np.outer(n2, k2v + 1) / N2
    w2re = np.cos(a2).astype(np.float32)
    w2im = (-np.sin(a2)).astype(np.float32)
    w2ure = np.cos(a2u).astype(np.float32)
    w2uim = (-np.sin(a2u)).astype(np.float32)

    # wide stationaries [64, 128]: rows p = 32g + n2 (g = rhs group),
    # cols p_out = 32G + k2.  Nonzero blocks:
    #   up (direct):   (g0,G0) k1 = jm      ; (g1,G1) k1 = 32+jm
    #   low (conj):    (g1,G2) k1' = 96-jm  ; (g0,G3) k1' = 128-jm
    def wide(up_blk, low_blk):
        s = np.zeros((64, 128), np.float32)
        s[0:32, 0:32] = up_blk
        s[32:64, 32:64] = up_blk
        s[32:64, 64:96] = low_blk
        s[0:32, 96:128] = low_blk
        return s

    return dict(
        g_mat=g_mat, h_mat=h_mat, t_re=t_re, t_im=t_im,
        su_a=wide(w2re, w2ure),      # C_re <- Bd_re
        su_b=wide(-w2im, w2uim),     # C_re <- Bd_im
        su_c=wide(w2im, w2uim),      # C_im <- Bd_re
        su_d=wide(w2re, -w2ure),     # C_im <- Bd_im
        w2re=w2re, w2im=w2im, nw2im=(-w2im).astype(np.float32).copy(),
    )


CONST_SHAPES = {
    "g_mat": (128, 65), "h_mat": (128, 65),
    "t_re": (65, 32), "t_im": (65, 32),
    "su_a": (64, 128), "su_b": (64, 128), "su_c": (64, 128), "su_d": (64, 128),
    "w2re": (32, 32), "w2im": (32, 32), "nw2im": (32, 32),
}

# constants that feed the PE as stationaries use the f32r streaming format
MM_CONSTS = {"g_mat", "h_mat", "su_a", "su_b", "su_c", "su_d",
             "w2re", "w2im", "nw2im"}


def _hand_ap(base_ap, rel_off, dims):
    return bass.AP(tensor=base_ap.tensor, offset=base_ap.offset + rel_off,
                   ap=[list(d) for d in dims])


# ---------------------------------------------------------------- program
def build_program(use_f32r=True):
    nc = bacc.Bacc("TRN2", target_bir_lowering=False, debug=False)

    MMDT = FP32R if use_f32r else FP32
    x_in = nc.dram_tensor("x", [BPER, N, M], MMDT, kind="ExternalInput")
    out_re = nc.dram_tensor("out_re", [BPER, N, M], FP32, kind="ExternalOutput")
    out_im = nc.dram_tensor("out_im", [BPER, N, M], FP32, kind="ExternalOutput")
    cin = {k: nc.dram_tensor(k, list(v), MMDT if k in MM_CONSTS else FP32,
                             kind="ExternalInput")
           for k, v in CONST_SHAPES.items()}
    # DRAM bounce: [b][n2][q][m], n2-major so phase-2 reads are 4KiB runs
    bdram = {c: nc.dram_tensor(f"bdram_{c}", [BPER, N2, QROWS, M], MMDT,
                               kind="Internal")
             for c in ("re", "im")}

    with tile.TileContext(nc) as tc, ExitStack() as ctx:
        cpool = ctx.enter_context(tc.tile_pool(name="consts", bufs=1))
        ct = {}
        for k, shp in CONST_SHAPES.items():
            ct[k] = cpool.tile(list(shp), MMDT if k in MM_CONSTS else FP32,
                               tag=k, name=f"ct_{k}")
            nc.sync.dma_start(ct[k][:], cin[k].ap())

        x_pool = ctx.enter_context(tc.tile_pool(name="x", bufs=2))
        a_psum = ctx.enter_context(tc.tile_pool(name="apsum", bufs=1, space="PSUM"))
        c_psum = ctx.enter_context(tc.tile_pool(name="cpsum", bufs=2, space="PSUM"))
        tw_pool = ctx.enter_context(tc.tile_pool(name="tw", bufs=1))
        b_pool = ctx.enter_context(tc.tile_pool(name="b", bufs=2))
        nyq_pool = ctx.enter_context(tc.tile_pool(name="nyq", bufs=2))
        bd_pool = ctx.enter_context(tc.tile_pool(name="bd", bufs=2))
        cs_pool = ctx.enter_context(tc.tile_pool(name="cs", bufs=2))
        cs64_pool = ctx.enter_context(tc.tile_pool(name="cs64", bufs=2))

        # ================= phase 1: stage-1 + twiddle + bounce write ======
        def phase1(b):
            X = x_pool.tile([128, N2 * M], MMDT, tag="X")
            nc.sync.dma_start(X[:], x_in.ap()[b])

            b_re_t = b_pool.tile([QROWS, N2 * M], MMDT, tag="b_re")
            b_im_t = b_pool.tile([QROWS, N2 * M], MMDT, tag="b_im")
            b_re, b_im = b_re_t[:], b_im_t[:]

            for fg in range(4):
                fs = slice(1024 * fg, 1024 * fg + 1024)
                a_re = a_psum.tile([QROWS, 1024], FP32, tag="a_re")
                a_im = a_psum.tile([QROWS, 1024], FP32, tag="a_im")
                for cc in (0, 1):
                    cw = slice(512 * cc, 512 * cc + 512)
                    rhs = X[:, 1024 * fg + 512 * cc: 1024 * fg + 512 * cc + 512]
                    nc.tensor.matmul(a_re[:, cw], ct["g_mat"][:], rhs,
                                     start=True, stop=True)
                    nc.tensor.matmul(a_im[:, cw], ct["h_mat"][:], rhs,
                                     start=True, stop=True)

                # compact twiddle [65, 8] broadcast over m via step-0 AP
                n2s = slice(8 * fg, 8 * fg + 8)
                tre_b = ct["t_re"][:, n2s].unsqueeze(2).broadcast_to(
                    [QROWS, 8, 128])
                tim_b = ct["t_im"][:, n2s].unsqueeze(2).broadcast_to(
                    [QROWS, 8, 128])
                a_re3 = a_re[:].rearrange("p (a m) -> p a m", m=128)
                a_im3 = a_im[:].rearrange("p (a m) -> p a m", m=128)
                p1 = tw_pool.tile([QROWS, 1024], FP32, tag="p1")
                p2 = tw_pool.tile([QROWS, 1024], FP32, tag="p2")
                p13 = p1[:].rearrange("p (a m) -> p a m", m=128)
                p23 = p2[:].rearrange("p (a m) -> p a m", m=128)
                nc.vector.tensor_mul(p13, a_re3, tre_b)
                nc.vector.tensor_mul(p23, a_im3, tim_b)
                nc.gpsimd.tensor_sub(b_re[:, fs], p1[:], p2[:])
                p3 = tw_pool.tile([QROWS, 1024], FP32, tag="p3")
                p4 = tw_pool.tile([QROWS, 1024], FP32, tag="p4")
                p33 = p3[:].rearrange("p (a m) -> p a m", m=128)
                p43 = p4[:].rearrange("p (a m) -> p a m", m=128)
                nc.vector.tensor_mul(p33, a_re3, tim_b)
                nc.vector.tensor_mul(p43, a_im3, tre_b)
                nc.gpsimd.tensor_add(b_im[:, fs], p3[:], p4[:])

            # bounce write: B[p = k1, f = n2*128+m] -> bdram[b, n2, q=k1, m]
            for comp, bsb in (("re", b_re), ("im", b_im)):
                dst = _hand_ap(bdram[comp].ap(), b * N2 * BD_ROW,
                               [[M, QROWS], [BD_ROW, N2], [1, M]])
                nc.sync.dma_start(dst, bsb)

        # ================= phase 2: load + stage-2 + out ==================
        def phase2(b):
            bd_re = bd_pool.tile([64, 4096], MMDT, tag="bd_re")
            bd_im = bd_pool.tile([64, 4096], MMDT, tag="bd_im")
            n32_re = nyq_pool.tile([32, 128], MMDT, tag="n32_re")
            n32_im = nyq_pool.tile([32, 128], MMDT, tag="n32_im")
            for comp, bd, n32 in (("re", bd_re, n32_re), ("im", bd_im, n32_im)):
                dram = bdram[comp].ap()
                for g in (0, 1):
                    src = _hand_ap(dram, b * N2 * BD_ROW + 32 * g * M,
                                   [[BD_ROW, 32], [1, 32 * M]])
                    nc.scalar.dma_start(bd[:][32 * g:32 * g + 32, :], src)
                src32 = _hand_ap(dram, b * N2 * BD_ROW + 64 * M,
                                 [[BD_ROW, 32], [1, M]])
                nc.scalar.dma_start(n32[:], src32)

            # Nyquist outputs: rows k = 128*k2 + 64 (borrows a cpsum slot)
            c64 = c_psum.tile([128, 512], FP32, tag="c_re")
            nc.tensor.matmul(c64[0:32, 0:128], ct["w2re"][:], n32_re[:],
                             start=True, stop=False)
            nc.tensor.matmul(c64[0:32, 0:128], ct["nw2im"][:], n32_im[:],
                             start=False, stop=True)
            nc.tensor.matmul(c64[0:32, 128:256], ct["w2im"][:], n32_re[:],
                             start=True, stop=False)
            nc.tensor.matmul(c64[0:32, 128:256], ct["w2re"][:], n32_im[:],
                             start=False, stop=True)
            cs64_re = cs64_pool.tile([32, 128], FP32, tag="cs64_re")
            cs64_im = cs64_pool.tile([32, 128], FP32, tag="cs64_im")
            nc.scalar.copy(cs64_re[:], c64[0:32, 0:128])
            nc.scalar.copy(cs64_im[:], c64[0:32, 128:256])
            for dram_t, cs_t in ((out_re, cs64_re), (out_im, cs64_im)):
                dst = _hand_ap(dram_t.ap(), b * N * M + 64 * M,
                               [[128 * M, 32], [1, M]])
                nc.sync.dma_start(dst, cs_t[:])

            # main stage 2, per quarter (jm = 8h + jml, jml = 0..7)
            for h in range(4):
                cs_re = cs_pool.tile([128, 1024], FP32, tag="cs_re")
                cs_im = cs_pool.tile([128, 1024], FP32, tag="cs_im")
                # group matmuls by stationary across the chunk pair to
                # halve PE weight reloads; psum pairs use the 2 pool slots
                cres, cims, rres, rims = [], [], [], []
                for cc in range(2):
                    ch = slice(512 * (2 * h + cc), 512 * (2 * h + cc) + 512)
                    rres.append(bd_re[:][:, ch])
                    rims.append(bd_im[:][:, ch])
                    cres.append(c_psum.tile([128, 512], FP32, tag="c_re",
                                            name=f"c_re_{h}_{cc}"))
                    cims.append(c_psum.tile([128, 512], FP32, tag="c_im",
                                            name=f"c_im_{h}_{cc}"))
                for cc in range(2):
                    nc.tensor.matmul(cres[cc][:], ct["su_a"][:], rres[cc],
                                     start=True, stop=False)
                for cc in range(2):
                    nc.tensor.matmul(cres[cc][:], ct["su_b"][:], rims[cc],
                                     start=False, stop=True)
                for cc in range(2):
                    nc.tensor.matmul(cims[cc][:], ct["su_c"][:], rres[cc],
                                     start=True, stop=False)
                for cc in range(2):
                    nc.tensor.matmul(cims[cc][:], ct["su_d"][:], rims[cc],
                                     start=False, stop=True)
                for cc in range(2):
                    cw = slice(512 * cc, 512 * cc + 512)
                    nc.scalar.copy(cs_re[:, cw], cres[cc][:])
                    nc.scalar.copy(cs_im[:, cw], cims[cc][:])

                # out rows: p = 32G + k2, f = jml*128 + m
                #   G0: 128k2 + jm      G1: 128k2 + 32 + jm
                #   G2: 128k2 + 96 - jm G3: 128k2 + 128 - jm (jm=0 dead)
                for dram_t, cs_t in ((out_re, cs_re), (out_im, cs_im)):
                    dap = dram_t.ap()
                    base = b * N * M
                    dst = _hand_ap(dap, base + 1024 * h,
                                   [[32 * M, 2], [128 * M, 32], [1, 1024]])
                    nc.sync.dma_start(dst, cs_t[0:64, :])
                    dst = _hand_ap(dap, base + 96 * M - 1024 * h,
                                   [[128 * M, 32], [-M, 8], [1, M]])
                    nc.sync.dma_start(dst, cs_t[64:96, :])
                    if h == 0:
                        dst = _hand_ap(dap, base + 128 * M - M,
                                       [[128 * M, 32], [-M, 7], [1, M]])
                        nc.sync.dma_start(dst, cs_t[96:128, M:])
                    else:
                        dst = _hand_ap(dap, base + 128 * M - 1024 * h,
                                       [[128 * M, 32], [-M, 8], [1, M]])
                        nc.sync.dma_start(dst, cs_t[96:128, :])

        # interleave with a lag so phase-2(b) overlaps phase-1(b+LAG)
        LAG = BPER  # sequential phases scheduled best
        for b in range(BPER + LAG):
            if b < BPER:
                phase1(b)
            if b >= LAG:
                phase2(b - LAG)

    nc.compile()
    return nc


_CACHE = {}


def _get_program():
    if "nc" not in _CACHE:
        _CACHE["nc"] = build_program()
        _CACHE["consts"] = make_consts()
    return _CACHE["nc"], _CACHE["consts"]


_LAST = {}


def _run(x: np.ndarray, trace: bool = False):
    x = np.ascontiguousarray(np.asarray(x, dtype=np.float32))
    assert x.shape == (B_FULL, N, M)
    nc, consts = _get_program()
    in_maps = []
    for c in range(NCORES):
        m = {"x": np.ascontiguousarray(x[c * BPER:(c + 1) * BPER])}
        m.update(consts)
        in_maps.append(m)
    bres = run_bass_kernel_spmd(nc, in_maps, list(range(NCORES)), trace=trace)
    _LAST["results"] = bres
    res = bres.results
    re = np.concatenate([res[c]["out_re"] for c in range(NCORES)], axis=0)
    im = np.concatenate([res[c]["out_im"] for c in range(NCORES)], axis=0)
    return re, im


def kernel(x: np.ndarray):
    """x: [64, 4096, 128] fp32 -> (re, im) each [64, 4096, 128] fp32."""
    return _run(x, trace=False)



# revision 9
# speedup vs baseline: 1.8331x; 1.8331x over previous
"""Trainium2 Bass kernel: batched FFT along axis 1 of x[64, 4096, 128] (fp32),
returning (real, imag) parts.  8-core data-parallel over the batch axis.

Algorithm (per core, 8 batches, fp16 internal precision):
4-step Cooley-Tukey with N = N1*N2 = 128*32, n = 32*n1 + n2, k = 128*k2 + k1:

    X[128*k2 + k1] = sum_n2 W32[n2,k2] * (W4096[n2*k1] * sum_n1 W128[n1,k1] * x)

Phase 1: stage-1 DFT-128 over n1 with the twiddle W4096[n2*k1] FOLDED into
  32 per-n2 stationary pairs P/Q[n1, q] (q = k1 = 0..64; cols 65..95 replicate
  the Nyquist row so it rides along for free).  Moving operand is x[n1, (b,m)]
  per n2 -> PSUM B[q, (b,m)].  No separate vector-engine twiddle pass.
Phase T: evictions assemble B as [q, (b, m, n2)] fp16 in SBUF; one DVE
  StreamTranspose per (comp, half, b) swaps the q<->n2 axes in 32x32 blocks
  (n2 is the innermost free dim), yielding Bd[32g+n2, (b, m, jm)] on-chip.
  This replaces the baseline's 34 MiB DRAM bounce round-trip entirely.
Phase 2: stage-2 DFT-32 over n2 with the wide [64,128] conjugate-packing
  stationaries (direct rows k1 = 32g+jm and Hermitian rows k1' = 128-q in one
  pass).  Moving free dim is (b, je, m) with jm = 2*jp+je, so fp16 output DMA
  runs are 512B (full DMA rate).  Nyquist rows use bd partitions 64..95.
"""

import numpy as np
from contextlib import ExitStack

import concourse.bacc as bacc
import concourse.bass as bass
import concourse.mybir as mybir
import concourse.tile as tile
from concourse.bass_utils import run_bass_kernel_spmd

N = 4096
N1, N2 = 128, 32
M = 128
B_FULL = 64
NCORES = 8
BPER = B_FULL // NCORES  # 8 batches per core

FP16 = mybir.dt.float16
FP32 = mybir.dt.float32

QS = 96          # stationary cols: q=0..63 direct, 64..95 nyquist-replicated


# ---------------------------------------------------------------- constants
def make_consts():
    # phase-1 folded stationaries: PQ[n1, n2*192 + c*96 + q]
    n1 = np.arange(N1)
    qv = np.minimum(np.arange(QS), 64)          # cols 64..95 all = nyquist 64
    pq = np.zeros((N1, N2 * 2 * QS), np.float32)
    for n2 in range(N2):
        theta = 2 * np.pi * np.outer(32 * n1 + n2, qv) / N
        pq[:, n2 * 192 + 0 * 96 : n2 * 192 + 0 * 96 + 96] = np.cos(theta)
        pq[:, n2 * 192 + 1 * 96 : n2 * 192 + 1 * 96 + 96] = -np.sin(theta)

    # phase-2 wide stationaries [64, 128] x 4 (SA | SB | SC | SD)
    n2v = np.arange(N2)
    k2v = np.arange(N2)
    a2 = 2 * np.pi * np.outer(n2v, k2v) / N2
    a2u = 2 * np.pi * np.outer(n2v, k2v + 1) / N2
    c, s = np.cos(a2), -np.sin(a2)
    cu, su_ = np.cos(a2u), -np.sin(a2u)

    def wide(up, low):
        z = np.zeros((64, 128), np.float32)
        z[0:32, 0:32] = up
        z[32:64, 32:64] = up
        z[32:64, 64:96] = low
        z[0:32, 96:128] = low
        return z

    su = np.concatenate(
        [wide(c, cu), wide(-s, su_), wide(s, su_), wide(c, -cu)], axis=1
    )  # [64, 512]

    # nyquist stationaries, staged at partitions 64..95 so the stationary
    # base partition matches the bd[64:96] moving slice
    ny = np.zeros((QS, 128), np.float32)
    ny[64:96] = np.concatenate([c, -s, s, c], axis=1)

    return {
        "pq": pq.astype(np.float16),
        "su": su.astype(np.float16),
        "ny": ny.astype(np.float16),
    }


def _hand_ap(base_ap, rel_off, dims):
    return bass.AP(tensor=base_ap.tensor, offset=base_ap.offset + rel_off,
                   ap=[list(d) for d in dims])


# ---------------------------------------------------------------- program
def build_program():
    nc = bacc.Bacc("TRN2", target_bir_lowering=False, debug=False)

    x_in = nc.dram_tensor("x", [BPER, N, M], FP16, kind="ExternalInput")
    # raw C-tile dumps; the (G,k2,c,b,je,m) -> [c,b,k,m] unscramble happens on
    # the host (pure relabeling).  outd[h, jp, po, (c,b,je,m)]
    outd = nc.dram_tensor("outd", [2, 16, 128, 2048], FP16,
                          kind="ExternalOutput")
    nyd = nc.dram_tensor("nyd", [2, 32, 1024], FP16, kind="ExternalOutput")
    pq_in = nc.dram_tensor("pq", [N1, N2 * 2 * QS], FP16, kind="ExternalInput")
    su_in = nc.dram_tensor("su", [64, 512], FP16, kind="ExternalInput")
    ny_in = nc.dram_tensor("ny", [QS, 128], FP16, kind="ExternalInput")

    with tile.TileContext(nc) as tc, ExitStack() as ctx:
        cpool = ctx.enter_context(tc.tile_pool(name="consts", bufs=1))
        ct_pq = cpool.tile([N1, N2 * 2 * QS], FP16, tag="pq", name="ct_pq")
        ct_su = cpool.tile([64, 512], FP16, tag="su", name="ct_su")
        ct_ny = cpool.tile([QS, 128], FP16, tag="ny", name="ct_ny")
        nc.sync.dma_start(ct_pq[:], pq_in.ap())
        nc.sync.dma_start(ct_su[:], su_in.ap())
        nc.sync.dma_start(ct_ny[:], ny_in.ap())

        x_pool = ctx.enter_context(tc.tile_pool(name="xp", bufs=2))
        a_psum = ctx.enter_context(tc.tile_pool(name="aps", bufs=2, space="PSUM"))
        c_psum = ctx.enter_context(tc.tile_pool(name="cps", bufs=1, space="PSUM"))
        b_pool = ctx.enter_context(tc.tile_pool(name="bp", bufs=1))
        bd_pool = ctx.enter_context(tc.tile_pool(name="bdp", bufs=3))
        cs_pool = ctx.enter_context(tc.tile_pool(name="csp", bufs=2))
        ny_pool = ctx.enter_context(tc.tile_pool(name="nyp", bufs=2))

        # eviction engine rotation (PSUM access: ACT and DVE only)
        ev_engines = [nc.scalar, nc.scalar, nc.vector]

        def phase1(h, b_sb):
            """b-half h: batches 4h..4h+3 -> B[comp] tiles [96, (b4, m, n2)]."""
            evi = 0
            for eb in range(4):          # batch within half
                b = 4 * h + eb
                xt = x_pool.tile([N1, N2 * M], FP16, tag="x", name=f"xt_{b}")
                src = _hand_ap(x_in.ap(), b * N * M,
                               [[N2 * M, N1], [1, N2 * M]])
                nc.sync.dma_start(xt[:], src)
                for o in range(4):       # n2 octet
                    for comp in range(2):
                        a = a_psum.tile([QS, 1024], FP32, tag="a",
                                        name=f"a_{b}_{o}_{comp}")
                        for j in range(8):
                            n2 = 8 * o + j
                            stat = ct_pq[:, n2 * 192 + comp * 96:
                                         n2 * 192 + comp * 96 + 96]
                            mov = xt[:, n2 * M:(n2 + 1) * M]
                            nc.tensor.matmul(a[:, j * 128:(j + 1) * 128],
                                             stat, mov, start=True, stop=True)
                        # evict [96, (n2_octet, m)] -> B[96, (b, m, n2)] slice
                        dst = b_sb[comp][:].rearrange(
                            "q (b m n) -> q b m n", b=4, m=M, n=N2)[
                            :, eb, :, 8 * o:8 * o + 8]
                        srcp = a[:].rearrange("q (n m) -> q m n", n=8, m=M)
                        eng = ev_engines[evi % len(ev_engines)]
                        evi += 1
                        if eng is nc.scalar:
                            eng.copy(dst, srcp)
                        else:
                            eng.tensor_copy(dst, srcp)
                        del eng

        def transpose_half(h, b_sb, bd):
            """StreamTranspose q<->n2 per b-chunk: B[96,(b,m,n2)] ->
            Bd[96=(g,n2 | nyq), (b, m, jm)]."""
            for comp in range(2):
                for eb in range(4):
                    fs = slice(eb * (M * N2), (eb + 1) * (M * N2))
                    nc.vector.transpose(bd[comp][:, fs], b_sb[comp][:, fs])

        def phase2(h, bd):
            for jp in range(16):
                cre = c_psum.tile([128, 1024], FP32, tag="cre",
                                  name=f"cre_{h}_{jp}")
                cim = c_psum.tile([128, 1024], FP32, tag="cim",
                                  name=f"cim_{h}_{jp}")
                # moving free limited to 512 (one PSUM bank) per matmul:
                # split the (b4, je, m) = 1024 free dim into b-pairs
                for bq in range(2):
                    movs = []
                    for comp in range(2):
                        mv = bd[comp][0:64, :].rearrange(
                            "p (b m j) -> p b j m", b=4, m=M, j=N2)[
                            :, 2 * bq:2 * bq + 2, 2 * jp:2 * jp + 2, :]
                        movs.append(mv)
                    cw = slice(512 * bq, 512 * bq + 512)
                    nc.tensor.matmul(cre[:, cw], ct_su[:, 0:128], movs[0],
                                     start=True, stop=False)
                    nc.tensor.matmul(cre[:, cw], ct_su[:, 128:256], movs[1],
                                     start=False, stop=True)
                    nc.tensor.matmul(cim[:, cw], ct_su[:, 256:384], movs[0],
                                     start=True, stop=False)
                    nc.tensor.matmul(cim[:, cw], ct_su[:, 384:512], movs[1],
                                     start=False, stop=True)

                csb = cs_pool.tile([128, 2048], FP16, tag="csb",
                                   name=f"csb_{h}_{jp}")
                nc.scalar.copy(csb[:, 0:1024], cre[:])
                nc.scalar.copy(csb[:, 1024:2048], cim[:])

                # contiguous dump: outd[h, jp, po, (c, b, je, m)]
                dst = _hand_ap(outd.ap(), (h * 16 + jp) * 128 * 2048,
                               [[2048, 128], [1, 2048]])
                nc.sync.dma_start(dst, csb[:])

        def nyquist(h, bd):
            cny = c_psum.tile([128, 1024], FP32, tag="cre", name=f"ny_{h}")
            movs = []
            for comp in range(2):
                mv = bd[comp][64:96, :].rearrange(
                    "p (b m j) -> p b m j", b=4, m=M, j=N2)[:, :, :, 0]
                movs.append(mv)
            nc.tensor.matmul(cny[0:32, 0:512], ct_ny[64:96, 0:32], movs[0],
                             start=True, stop=False)
            nc.tensor.matmul(cny[0:32, 0:512], ct_ny[64:96, 32:64], movs[1],
                             start=False, stop=True)
            nc.tensor.matmul(cny[0:32, 512:1024], ct_ny[64:96, 64:96], movs[0],
                             start=True, stop=False)
            nc.tensor.matmul(cny[0:32, 512:1024], ct_ny[64:96, 96:128], movs[1],
                             start=False, stop=True)
            nsb = ny_pool.tile([32, 1024], FP16, tag="nsb", name=f"nsb_{h}")
            nc.scalar.copy(nsb[:], cny[0:32, :])
            dst = _hand_ap(nyd.ap(), h * 32 * 1024, [[1024, 32], [1, 1024]])
            nc.sync.dma_start(dst, nsb[:])

        # ---------------- pipeline ----------------
        b_sb = {}
        bd = {}
        for h in range(2):
            b_sb[h] = {c: b_pool.tile([QS, 4 * M * N2], FP16, tag=f"b{c}",
                                      name=f"bsb_{h}_{c}") for c in range(2)}
            bd[h] = {c: bd_pool.tile([QS, 4 * M * N2], FP16, tag="bd",
                                     name=f"bd_{h}_{c}") for c in range(2)}

        # emission order: phase2(0) precedes transpose_half(1) so the WAR on
        # the aliased bd buffer (bufs=3: bd[1][1] reuses bd[0][0]'s space)
        # orders T(1,c1) after phase2(0)'s reads; T(1,c0) has a fresh buffer
        # and overlaps phase2(0) on the DVE.
        phase1(0, b_sb[0])
        transpose_half(0, b_sb[0], bd[0])
        phase1(1, b_sb[1])
        phase2(0, bd[0])
        nyquist(0, bd[0])
        transpose_half(1, b_sb[1], bd[1])
        phase2(1, bd[1])
        nyquist(1, bd[1])

    nc.compile()
    return nc


_CACHE = {}


def _get_program():
    if "nc" not in _CACHE:
        _CACHE["nc"] = build_program()
        _CACHE["consts"] = make_consts()
    return _CACHE["nc"], _CACHE["consts"]


_LAST = {}


def _run(x: np.ndarray, trace: bool = False):
    x = np.asarray(x)
    assert x.shape == (B_FULL, N, M)
    x16 = np.ascontiguousarray(x.astype(np.float16))
    nc, consts = _get_program()
    in_maps = []
    for c in range(NCORES):
        m = {"x": np.ascontiguousarray(x16[c * BPER:(c + 1) * BPER])}
        m.update(consts)
        in_maps.append(m)
    bres = run_bass_kernel_spmd(nc, in_maps, list(range(NCORES)), trace=trace)
    _LAST["results"] = bres
    res = bres.results
    out = np.empty((2, B_FULL, N, M), np.float32)
    k1m, valid = _k1_map()
    k1f = k1m.reshape(-1)[valid]
    for core in range(NCORES):
        scr = res[core]["outd"].astype(np.float32)
        nyq = res[core]["nyd"].astype(np.float32)
        # scr: [h, jp, po, f] -> (h jp G k2 c b je m)
        s = scr.reshape(2, 16, 4, 32, 2, 4, 2, M)
        p = s.transpose(4, 0, 5, 1, 2, 6, 3, 7)   # c h b jp G je k2 m
        flat = p.reshape(2, 2, 4, 16 * 4 * 2, 32, M)
        xv = out[:, core * BPER:(core + 1) * BPER].reshape(2, 2, 4, 32, 128, M)
        xv[:, :, :, :, k1f, :] = flat[:, :, :, valid].transpose(0, 1, 2, 4, 3, 5)
        ny = nyq.reshape(2, 32, 2, 4, M).transpose(2, 0, 3, 1, 4)  # c h b k2 m
        xv[:, :, :, :, 64, :] = ny
    return out[0], out[1]


def _k1_map():
    k1 = np.zeros((16, 4, 2), np.int64)
    jp = np.arange(16)[:, None]
    je = np.arange(2)[None, :]
    k1[:, 0, :] = 2 * jp + je
    k1[:, 1, :] = 32 + 2 * jp + je
    k1[:, 2, :] = 96 - 2 * jp - je
    k1[:, 3, :] = 128 - 2 * jp - je
    valid = (k1 < 128).reshape(-1)   # drop jp=0,G3,je=0 (k1=128 wrap dup)
    return k1, valid


def kernel(x: np.ndarray):
    """x: [64, 4096, 128] fp32 -> (re, im) each [64, 4096, 128] fp32."""
    return _run(x, trace=False)


# revision 10
# speedup vs baseline: 1.8959x; 1.0343x over previous
"""Trainium2 Bass kernel: batched FFT along axis 1 of x[64, 4096, 128] (fp32),
returning (real, imag) parts.  8-core data-parallel over the batch axis.

Algorithm (per core, 8 batches, fp16 internal precision):
4-step Cooley-Tukey with N = N1*N2 = 128*32, n = 32*n1 + n2, k = 128*k2 + k1:

    X[128*k2 + k1] = sum_n2 W32[n2,k2] * (W4096[n2*k1] * sum_n1 W128[n1,k1] * x)

Phase 1: stage-1 DFT-128 over n1 with the twiddle W4096[n2*k1] FOLDED into
  32 per-n2 stationary pairs P/Q[n1, q] (q = k1 = 0..64; cols 65..95 replicate
  the Nyquist row so it rides along for free).  Moving operand is x[n1, (b,m)]
  per n2 -> PSUM B[q, (b,m)].  No separate vector-engine twiddle pass.
Phase T: evictions assemble B as [q, (b, m, n2)] fp16 in SBUF; one DVE
  StreamTranspose per (comp, half, b) swaps the q<->n2 axes in 32x32 blocks
  (n2 is the innermost free dim), yielding Bd[32g+n2, (b, m, jm)] on-chip.
  This replaces the baseline's 34 MiB DRAM bounce round-trip entirely.
Phase 2: stage-2 DFT-32 over n2 with the wide [64,128] conjugate-packing
  stationaries (direct rows k1 = 32g+jm and Hermitian rows k1' = 128-q in one
  pass).  Moving free dim is (b, je, m) with jm = 2*jp+je, so fp16 output DMA
  runs are 512B (full DMA rate).  Nyquist rows use bd partitions 64..95.
"""

import numpy as np
from contextlib import ExitStack

import concourse.bacc as bacc
import concourse.bass as bass
import concourse.mybir as mybir
import concourse.tile as tile
from concourse.bass_utils import run_bass_kernel_spmd

N = 4096
N1, N2 = 128, 32
M = 128
B_FULL = 64
NCORES = 8
BPER = B_FULL // NCORES  # 8 batches per core

FP16 = mybir.dt.float16
FP32 = mybir.dt.float32

QS = 96          # stationary cols: q=0..63 direct, 64..95 nyquist-replicated


# ---------------------------------------------------------------- constants
def make_consts():
    # phase-1 folded stationaries: PQ[n1, n2*192 + c*96 + q]
    n1 = np.arange(N1)
    qv = np.minimum(np.arange(QS), 64)          # cols 64..95 all = nyquist 64
    pq = np.zeros((N1, N2 * 2 * QS), np.float32)
    for n2 in range(N2):
        theta = 2 * np.pi * np.outer(32 * n1 + n2, qv) / N
        pq[:, n2 * 192 + 0 * 96 : n2 * 192 + 0 * 96 + 96] = np.cos(theta)
        pq[:, n2 * 192 + 1 * 96 : n2 * 192 + 1 * 96 + 96] = -np.sin(theta)

    # phase-2 wide stationaries [64, 128] x 4 (SA | SB | SC | SD)
    n2v = np.arange(N2)
    k2v = np.arange(N2)
    a2 = 2 * np.pi * np.outer(n2v, k2v) / N2
    a2u = 2 * np.pi * np.outer(n2v, k2v + 1) / N2
    c, s = np.cos(a2), -np.sin(a2)
    cu, su_ = np.cos(a2u), -np.sin(a2u)

    def wide(up, low):
        z = np.zeros((64, 128), np.float32)
        z[0:32, 0:32] = up
        z[32:64, 32:64] = up
        z[32:64, 64:96] = low
        z[0:32, 96:128] = low
        return z

    su = np.concatenate(
        [wide(c, cu), wide(-s, su_), wide(s, su_), wide(c, -cu)], axis=1
    )  # [64, 512]

    # nyquist stationaries, staged at partitions 64..95 so the stationary
    # base partition matches the bd[64:96] moving slice
    ny = np.zeros((QS, 128), np.float32)
    ny[64:96] = np.concatenate([c, -s, s, c], axis=1)

    return {
        "pq": pq.astype(np.float16),
        "su": su.astype(np.float16),
        "ny": ny.astype(np.float16),
    }


def _hand_ap(base_ap, rel_off, dims):
    return bass.AP(tensor=base_ap.tensor, offset=base_ap.offset + rel_off,
                   ap=[list(d) for d in dims])


# ---------------------------------------------------------------- program
def build_program():
    nc = bacc.Bacc("TRN2", target_bir_lowering=False, debug=False)

    x_in = nc.dram_tensor("x", [BPER, N, M], FP16, kind="ExternalInput")
    # raw C-tile dumps; the (G,k2,c,b,je,m) -> [c,b,k,m] unscramble happens on
    # the host (pure relabeling).  outd[h, jp, po, (c,b,je,m)]
    outd = nc.dram_tensor("outd", [2, 16, 128, 2048], FP16,
                          kind="ExternalOutput")
    nyd = nc.dram_tensor("nyd", [2, 32, 1024], FP16, kind="ExternalOutput")
    pq_in = nc.dram_tensor("pq", [N1, N2 * 2 * QS], FP16, kind="ExternalInput")
    su_in = nc.dram_tensor("su", [64, 512], FP16, kind="ExternalInput")
    ny_in = nc.dram_tensor("ny", [QS, 128], FP16, kind="ExternalInput")

    with tile.TileContext(nc) as tc, ExitStack() as ctx:
        cpool = ctx.enter_context(tc.tile_pool(name="consts", bufs=1))
        ct_pq = cpool.tile([N1, N2 * 2 * QS], FP16, tag="pq", name="ct_pq")
        ct_su = cpool.tile([64, 512], FP16, tag="su", name="ct_su")
        ct_ny = cpool.tile([QS, 128], FP16, tag="ny", name="ct_ny")
        nc.sync.dma_start(ct_pq[:], pq_in.ap())
        nc.sync.dma_start(ct_su[:], su_in.ap())
        nc.sync.dma_start(ct_ny[:], ny_in.ap())

        x_pool = ctx.enter_context(tc.tile_pool(name="xp", bufs=2))
        a_psum = ctx.enter_context(tc.tile_pool(name="aps", bufs=2, space="PSUM"))
        c_psum = ctx.enter_context(tc.tile_pool(name="cps", bufs=2, space="PSUM"))
        b_pool = ctx.enter_context(tc.tile_pool(name="bp", bufs=1))
        bd_pool = ctx.enter_context(tc.tile_pool(name="bdp", bufs=3))
        cs_pool = ctx.enter_context(tc.tile_pool(name="csp", bufs=2))
        ny_pool = ctx.enter_context(tc.tile_pool(name="nyp", bufs=2))

        # B evictions all on ACT; DVE runs the StreamTransposes
        ev_engines = [nc.scalar]

        def phase1(h, b_sb):
            """b-half h: batches 4h..4h+3 -> B[comp] tiles [96, (b4, m, n2)]."""
            evi = 0
            for eb in range(4):          # batch within half
                b = 4 * h + eb
                xt = x_pool.tile([N1, N2 * M], FP16, tag="x", name=f"xt_{b}")
                src = _hand_ap(x_in.ap(), b * N * M,
                               [[N2 * M, N1], [1, N2 * M]])
                nc.sync.dma_start(xt[:], src)
                for o in range(4):       # n2 octet
                    for comp in range(2):
                        a = a_psum.tile([QS, 1024], FP32, tag="a",
                                        name=f"a_{b}_{o}_{comp}")
                        for j in range(8):
                            n2 = 8 * o + j
                            stat = ct_pq[:, n2 * 192 + comp * 96:
                                         n2 * 192 + comp * 96 + 96]
                            mov = xt[:, n2 * M:(n2 + 1) * M]
                            nc.tensor.matmul(a[:, j * 128:(j + 1) * 128],
                                             stat, mov, start=True, stop=True)
                        # evict [96, (n2_octet, m)] -> B[96, (b, m, n2)] slice
                        dst = b_sb[comp][:].rearrange(
                            "q (b m n) -> q b m n", b=4, m=M, n=N2)[
                            :, eb, :, 8 * o:8 * o + 8]
                        srcp = a[:].rearrange("q (n m) -> q m n", n=8, m=M)
                        eng = ev_engines[evi % len(ev_engines)]
                        evi += 1
                        if eng is nc.scalar:
                            eng.copy(dst, srcp)
                        else:
                            eng.tensor_copy(dst, srcp)
                        del eng

        def transpose_half(h, b_sb, bd):
            """StreamTranspose q<->n2 per b-chunk: B[96,(b,m,n2)] ->
            Bd[96=(g,n2 | nyq), (b, m, jm)]."""
            for comp in range(2):
                for eb in range(4):
                    fs = slice(eb * (M * N2), (eb + 1) * (M * N2))
                    nc.vector.transpose(bd[comp][:, fs], b_sb[comp][:, fs])

        def phase2(h, bd):
            for jp in range(16):
                csb = cs_pool.tile([128, 2048], FP16, tag="csb",
                                   name=f"csb_{h}_{jp}")
                # [128, 512] psum tiles (1 bank) with bufs=2 per tag so the
                # jp-chain double-buffers; cre evicts on ACT, cim on DVE
                for bq in range(2):
                    cre = c_psum.tile([128, 512], FP32, tag="cre",
                                      name=f"cre_{h}_{jp}_{bq}")
                    cim = c_psum.tile([128, 512], FP32, tag="cim",
                                      name=f"cim_{h}_{jp}_{bq}")
                    movs = []
                    for comp in range(2):
                        mv = bd[comp][0:64, :].rearrange(
                            "p (b m j) -> p b j m", b=4, m=M, j=N2)[
                            :, 2 * bq:2 * bq + 2, 2 * jp:2 * jp + 2, :]
                        movs.append(mv)
                    nc.tensor.matmul(cre[:], ct_su[:, 0:128], movs[0],
                                     start=True, stop=False)
                    nc.tensor.matmul(cre[:], ct_su[:, 128:256], movs[1],
                                     start=False, stop=True)
                    nc.tensor.matmul(cim[:], ct_su[:, 256:384], movs[0],
                                     start=True, stop=False)
                    nc.tensor.matmul(cim[:], ct_su[:, 384:512], movs[1],
                                     start=False, stop=True)
                    cw = slice(512 * bq, 512 * bq + 512)
                    cwi = slice(1024 + 512 * bq, 1024 + 512 * bq + 512)
                    nc.scalar.copy(csb[:, cw], cre[:])
                    nc.vector.tensor_copy(csb[:, cwi], cim[:])

                # contiguous dump: outd[h, jp, po, (c, b, je, m)]
                dst = _hand_ap(outd.ap(), (h * 16 + jp) * 128 * 2048,
                               [[2048, 128], [1, 2048]])
                nc.sync.dma_start(dst, csb[:])

        def nyquist(h, bd):
            cnr = c_psum.tile([128, 512], FP32, tag="cre", name=f"nyr_{h}")
            cni = c_psum.tile([128, 512], FP32, tag="cim", name=f"nyi_{h}")
            movs = []
            for comp in range(2):
                mv = bd[comp][64:96, :].rearrange(
                    "p (b m j) -> p b m j", b=4, m=M, j=N2)[:, :, :, 0]
                movs.append(mv)
            nc.tensor.matmul(cnr[0:32, :], ct_ny[64:96, 0:32], movs[0],
                             start=True, stop=False)
            nc.tensor.matmul(cnr[0:32, :], ct_ny[64:96, 32:64], movs[1],
                             start=False, stop=True)
            nc.tensor.matmul(cni[0:32, :], ct_ny[64:96, 64:96], movs[0],
                             start=True, stop=False)
            nc.tensor.matmul(cni[0:32, :], ct_ny[64:96, 96:128], movs[1],
                             start=False, stop=True)
            nsb = ny_pool.tile([32, 1024], FP16, tag="nsb", name=f"nsb_{h}")
            nc.scalar.copy(nsb[:, 0:512], cnr[0:32, :])
            nc.vector.tensor_copy(nsb[:, 512:1024], cni[0:32, :])
            dst = _hand_ap(nyd.ap(), h * 32 * 1024, [[1024, 32], [1, 1024]])
            nc.sync.dma_start(dst, nsb[:])

        # ---------------- pipeline ----------------
        b_sb = {}
        bd = {}
        for h in range(2):
            b_sb[h] = {c: b_pool.tile([QS, 4 * M * N2], FP16, tag=f"b{c}",
                                      name=f"bsb_{h}_{c}") for c in range(2)}
            bd[h] = {c: bd_pool.tile([QS, 4 * M * N2], FP16, tag="bd",
                                     name=f"bd_{h}_{c}") for c in range(2)}

        # emission order: phase2(0) precedes transpose_half(1) so the WAR on
        # the aliased bd buffer (bufs=3: bd[1][1] reuses bd[0][0]'s space)
        # orders T(1,c1) after phase2(0)'s reads; T(1,c0) has a fresh buffer
        # and overlaps phase2(0) on the DVE.
        phase1(0, b_sb[0])
        transpose_half(0, b_sb[0], bd[0])
        phase1(1, b_sb[1])
        phase2(0, bd[0])
        nyquist(0, bd[0])
        transpose_half(1, b_sb[1], bd[1])
        phase2(1, bd[1])
        nyquist(1, bd[1])

    nc.compile()
    return nc


_CACHE = {}


def _get_program():
    if "nc" not in _CACHE:
        _CACHE["nc"] = build_program()
        _CACHE["consts"] = make_consts()
    return _CACHE["nc"], _CACHE["consts"]


_LAST = {}


def _run(x: np.ndarray, trace: bool = False):
    x = np.asarray(x)
    assert x.shape == (B_FULL, N, M)
    x16 = np.ascontiguousarray(x.astype(np.float16))
    nc, consts = _get_program()
    in_maps = []
    for c in range(NCORES):
        m = {"x": np.ascontiguousarray(x16[c * BPER:(c + 1) * BPER])}
        m.update(consts)
        in_maps.append(m)
    bres = run_bass_kernel_spmd(nc, in_maps, list(range(NCORES)), trace=trace)
    _LAST["results"] = bres
    res = bres.results
    out = np.empty((2, B_FULL, N, M), np.float32)
    k1m, valid = _k1_map()
    k1f = k1m.reshape(-1)[valid]
    for core in range(NCORES):
        scr = res[core]["outd"].astype(np.float32)
        nyq = res[core]["nyd"].astype(np.float32)
        # scr: [h, jp, po, f] -> (h jp G k2 c b je m)
        s = scr.reshape(2, 16, 4, 32, 2, 4, 2, M)
        p = s.transpose(4, 0, 5, 1, 2, 6, 3, 7)   # c h b jp G je k2 m
        flat = p.reshape(2, 2, 4, 16 * 4 * 2, 32, M)
        xv = out[:, core * BPER:(core + 1) * BPER].reshape(2, 2, 4, 32, 128, M)
        xv[:, :, :, :, k1f, :] = flat[:, :, :, valid].transpose(0, 1, 2, 4, 3, 5)
        ny = nyq.reshape(2, 32, 2, 4, M).transpose(2, 0, 3, 1, 4)  # c h b k2 m
        xv[:, :, :, :, 64, :] = ny
    return out[0], out[1]


def _k1_map():
    k1 = np.zeros((16, 4, 2), np.int64)
    jp = np.arange(16)[:, None]
    je = np.arange(2)[None, :]
    k1[:, 0, :] = 2 * jp + je
    k1[:, 1, :] = 32 + 2 * jp + je
    k1[:, 2, :] = 96 - 2 * jp - je
    k1[:, 3, :] = 128 - 2 * jp - je
    valid = (k1 < 128).reshape(-1)   # drop jp=0,G3,je=0 (k1=128 wrap dup)
    return k1, valid


def kernel(x: np.ndarray):
    """x: [64, 4096, 128] fp32 -> (re, im) each [64, 4096, 128] fp32."""
    return _run(x, trace=False)


# revision 17
# speedup vs baseline: 2.5472x; 1.3435x over previous
"""Trainium2 Bass kernel: batched FFT along axis 1 of x[64, 4096, 128] (fp32),
returning (real, imag) parts.  8-core data-parallel over the batch axis.

Algorithm (per core, 8 batches): 4-step Cooley-Tukey, N = 128*32,
n = 32*n1 + n2, k = 128*k2 + k1:

    X[128*k2 + k1] = sum_n2 W32[n2,k2] * (W4096[n2*k1] * sum_n1 W128[n1,k1]*x)

Phase 1 (per batch b, m-range mr of 32): stage-1 DFT-128 over n1 with the
  twiddle FOLDED into per-n2 stationaries PQ[n1, (comp, k1=0..63)] -- both
  components packed into one 128-wide output so every PSUM partition carries
  real data.  One f=32 matmul per n2 fills a strided column slot of a PSUM
  tile a[128, (m32, n2-32)] (n2 innermost).
Evict: one ACT copy per PSUM tile -> fp16 B-slab [128=(c,g,jm), (b2,m,n2)].
Transpose: one DVE StreamTranspose per quarter (32x32 blocks) yields
  Bd[(c,g,n2), (b2, m, jm)] fp16 on-chip -- no DRAM bounce.
Phase 2 (per quarter, jp): stage-2 DFT-32 over n2 with wide [64,128]
  conjugate-packing stationaries (direct k1 = 32g+jm and Hermitian
  k1' = 128-q rows in one pass); fp16 moving, free dim (b2, je, m).
  C tiles are dumped verbatim to DRAM; the host relabels (pure data
  movement, no arithmetic).
Nyquist rows (k1 = 64) skip the transpose: X[128k2+64] is computed directly
  from x by a single-stage matmul accumulated over n2 in PSUM.
"""

import numpy as np
from contextlib import ExitStack

import concourse.bacc as bacc
import concourse.bass as bass
import concourse.mybir as mybir
import concourse.tile as tile
from concourse.bass_utils import run_bass_kernel_spmd

N = 4096
N1, N2 = 128, 32
M = 128
B_FULL = 64
NCORES = 8
BPER = B_FULL // NCORES  # 8 batches per core

FP16 = mybir.dt.float16
FP32 = mybir.dt.float32


# ---------------------------------------------------------------- constants
def make_consts():
    n1 = np.arange(N1)
    q = np.arange(64)
    # phase-1 folded stationaries: PQ[n1, n2*128 + c*64 + q], q = k1 = 0..63
    pq = np.zeros((N1, N2 * 128), np.float32)
    for n2 in range(N2):
        theta = 2 * np.pi * np.outer(32 * n1 + n2, q) / N
        pq[:, n2 * 128: n2 * 128 + 64] = np.cos(theta)
        pq[:, n2 * 128 + 64: n2 * 128 + 128] = -np.sin(theta)

    # nyquist single-stage stationaries: NY[n1, n2*64 + c*32 + k2]
    k2 = np.arange(N2)
    nyst = np.zeros((N1, N2 * 64), np.float32)
    for n2 in range(N2):
        th = 2 * np.pi * np.outer(32 * n1 + n2, 128 * k2 + 64) / N
        nyst[:, n2 * 64: n2 * 64 + 32] = np.cos(th)
        nyst[:, n2 * 64 + 32: n2 * 64 + 64] = -np.sin(th)

    # phase-2 wide stationaries [128, 512]: rows 0..63 pair with the
    # re-moving (bd[0:64]), rows 64..127 with the im-moving (bd[64:128]).
    n2v = np.arange(N2)
    a2 = 2 * np.pi * np.outer(n2v, k2) / N2
    a2u = 2 * np.pi * np.outer(n2v, k2 + 1) / N2
    c, s = np.cos(a2), -np.sin(a2)
    cu, su_ = np.cos(a2u), -np.sin(a2u)

    def wide(up, low):
        z = np.zeros((64, 128), np.float32)
        z[0:32, 0:32] = up
        z[32:64, 32:64] = up
        z[32:64, 64:96] = low
        z[0:32, 96:128] = low
        return z

    # stacked [128, 128] stationaries: rows 0..63 act on Bd_re (bd[0:64]),
    # rows 64..127 on Bd_im (bd[64:128]) -- one matmul per C component
    su = np.zeros((128, 256), np.float32)
    su[0:64, 0:128] = wide(c, cu)        # SA: C_re <- Bd_re
    su[64:128, 0:128] = wide(-s, su_)    # SB: C_re <- Bd_im
    su[0:64, 128:256] = wide(s, su_)     # SC: C_im <- Bd_re
    su[64:128, 128:256] = wide(c, -cu)   # SD: C_im <- Bd_im

    return {
        "pq": pq.astype(np.float16),
        "nyst": nyst.astype(np.float16),
        "su": su.astype(np.float16),
    }


def _hand_ap(base_ap, rel_off, dims):
    return bass.AP(tensor=base_ap.tensor, offset=base_ap.offset + rel_off,
                   ap=[list(d) for d in dims])


# ---------------------------------------------------------------- program
def build_program():
    nc = bacc.Bacc("TRN2", target_bir_lowering=False, debug=False)

    x_in = nc.dram_tensor("x", [BPER, N, M], FP16, kind="ExternalInput")
    # raw C-tile dumps: outd[q, jp, po, (c, b2, je, m)]
    outd = nc.dram_tensor("outd", [4, 16, 128, 1024], FP16,
                          kind="ExternalOutput")
    nyd = nc.dram_tensor("nyd", [4, 64, 256], FP16, kind="ExternalOutput")
    pq_in = nc.dram_tensor("pq", [N1, N2 * 128], FP16, kind="ExternalInput")
    nyst_in = nc.dram_tensor("nyst", [N1, N2 * 64], FP16,
                             kind="ExternalInput")
    su_in = nc.dram_tensor("su", [128, 256], FP16, kind="ExternalInput")

    with tile.TileContext(nc) as tc, ExitStack() as ctx:
        cpool = ctx.enter_context(tc.tile_pool(name="consts", bufs=1))
        ct_pq = cpool.tile([N1, N2 * 128], FP16, tag="pq", name="ct_pq")
        ct_ny = cpool.tile([N1, N2 * 64], FP16, tag="nyst", name="ct_ny")
        ct_su = cpool.tile([128, 256], FP16, tag="su", name="ct_su")
        nc.sync.dma_start(ct_pq[:], pq_in.ap())
        nc.sync.dma_start(ct_ny[:], nyst_in.ap())
        nc.sync.dma_start(ct_su[:], su_in.ap())

        x_pool = ctx.enter_context(tc.tile_pool(name="xp", bufs=2))
        a_psum = ctx.enter_context(tc.tile_pool(name="aps", bufs=4, space="PSUM"))
        c_psum = ctx.enter_context(tc.tile_pool(name="cps", bufs=2, space="PSUM"))
        b_pool = ctx.enter_context(tc.tile_pool(name="bp", bufs=2))
        bd_pool = ctx.enter_context(tc.tile_pool(name="bdp", bufs=3))
        cs_pool = ctx.enter_context(tc.tile_pool(name="csp", bufs=3))
        ny_pool = ctx.enter_context(tc.tile_pool(name="nyp", bufs=2))

        def phase1_quarter(qq, bq, nysb):
            """batches 2qq, 2qq+1 -> B-slab [128=(c,q64), (b2, m, n2)] fp16."""
            for eb in range(2):
                b = 2 * qq + eb
                xt = x_pool.tile([N1, N2 * M], FP16, tag="x", name=f"xt_{b}")
                src = _hand_ap(x_in.ap(), b * N * M,
                               [[N2 * M, N1], [1, N2 * M]])
                nc.sync.dma_start(xt[:], src)
                for ng in range(8):   # n2 quads, contiguous psum writes
                    a = a_psum.tile([128, 512], FP32, tag="a",
                                    name=f"a_{b}_{ng}")
                    for j in range(4):
                        n2 = 4 * ng + j
                        stat = ct_pq[:, n2 * 128:(n2 + 1) * 128]
                        mov = xt[:, n2 * M:(n2 + 1) * M]
                        nc.tensor.matmul(a[:, j * 128:(j + 1) * 128],
                                         stat, mov, start=True, stop=True)
                    # strided eviction performs the (n2, m) -> (m, n2) reorder
                    dst = bq[:].rearrange("p (b m n) -> p b m n",
                                          b=2, m=M, n=N2)[
                        :, eb, :, 4 * ng:4 * ng + 4]
                    srcp = a[:].rearrange("p (n m) -> p m n", n=4, m=M)
                    nc.scalar.copy(dst, srcp)
                # nyquist: single-stage, accumulate over n2 in PSUM
                any_ = a_psum.tile([128, 512], FP32, tag="a", name=f"any_{b}")
                for n2 in range(N2):
                    stat = ct_ny[:, n2 * 64:(n2 + 1) * 64]
                    mov = xt[:, n2 * M:(n2 + 1) * M]
                    nc.tensor.matmul(any_[0:64, 0:128], stat, mov,
                                     start=(n2 == 0), stop=(n2 == N2 - 1))
                nc.scalar.copy(nysb[:, eb * 128:(eb + 1) * 128],
                               any_[0:64, 0:128])

        def phase2_quarter(qq, bd):
            for jp in range(16):
                csb = cs_pool.tile([128, 1024], FP16, tag="csb",
                                   name=f"csb_{qq}_{jp}")
                cre = c_psum.tile([128, 512], FP32, tag="cre",
                                  name=f"cre_{qq}_{jp}")
                cim = c_psum.tile([128, 512], FP32, tag="cim",
                                  name=f"cim_{qq}_{jp}")
                bd4 = bd[:].rearrange("p (b m j) -> p b j m", b=2, m=M, j=N2)
                mov = bd4[:, :, 2 * jp:2 * jp + 2, :]
                nc.tensor.matmul(cre[:], ct_su[:, 0:128], mov,
                                 start=True, stop=True)
                nc.tensor.matmul(cim[:], ct_su[:, 128:256], mov,
                                 start=True, stop=True)
                nc.scalar.copy(csb[:, 0:512], cre[:])
                nc.vector.tensor_copy(csb[:, 512:1024], cim[:])
                dst = _hand_ap(outd.ap(), (qq * 16 + jp) * 128 * 1024,
                               [[1024, 128], [1, 1024]])
                nc.sync.dma_start(dst, csb[:])

        # ---------------- pipeline ----------------
        for qq in range(4):
            bq = b_pool.tile([128, 2 * M * N2], FP16, tag="b",
                             name=f"bq_{qq}")
            bd = bd_pool.tile([128, 2 * M * N2], FP16, tag="bd",
                              name=f"bd_{qq}")
            nysb = ny_pool.tile([64, 256], FP16, tag="nsb", name=f"nysb_{qq}")
            phase1_quarter(qq, bq, nysb)
            nc.vector.transpose(bd[:], bq[:])
            dstn = _hand_ap(nyd.ap(), qq * 64 * 256, [[256, 64], [1, 256]])
            nc.sync.dma_start(dstn, nysb[:])
            phase2_quarter(qq, bd)

    nc.compile()
    return nc


_CACHE = {}


def _get_program():
    if "nc" not in _CACHE:
        _CACHE["nc"] = build_program()
        _CACHE["consts"] = make_consts()
    return _CACHE["nc"], _CACHE["consts"]


_LAST = {}


def _k1_map():
    k1 = np.zeros((16, 4, 2), np.int64)
    jp = np.arange(16)[:, None]
    je = np.arange(2)[None, :]
    k1[:, 0, :] = 2 * jp + je
    k1[:, 1, :] = 32 + 2 * jp + je
    k1[:, 2, :] = 96 - 2 * jp - je
    k1[:, 3, :] = 128 - 2 * jp - je
    valid = (k1 < 128).reshape(-1)   # drop jp=0,G3,je=0 (k1=128 wrap dup)
    return k1, valid


def _run(x: np.ndarray, trace: bool = False):
    x = np.asarray(x)
    assert x.shape == (B_FULL, N, M)
    x16 = np.ascontiguousarray(x.astype(np.float16))
    nc, consts = _get_program()
    in_maps = []
    for c in range(NCORES):
        m = {"x": np.ascontiguousarray(x16[c * BPER:(c + 1) * BPER])}
        m.update(consts)
        in_maps.append(m)
    bres = run_bass_kernel_spmd(nc, in_maps, list(range(NCORES)), trace=trace)
    _LAST["results"] = bres
    res = bres.results
    out = np.empty((2, B_FULL, N, M), np.float32)
    k1m, valid = _k1_map()
    k1f = k1m.reshape(-1)[valid]
    for core in range(NCORES):
        scr = res[core]["outd"].astype(np.float32)
        nyq = res[core]["nyd"].astype(np.float32)
        # scr: [q, jp, po, f] -> (q jp G k2 c b2 je m)
        s = scr.reshape(4, 16, 4, 32, 2, 2, 2, M)
        p = s.transpose(4, 0, 5, 1, 2, 6, 3, 7)   # c q b2 jp G je k2 m
        flat = p.reshape(2, 4, 2, 16 * 4 * 2, 32, M)
        xv = out[:, core * BPER:(core + 1) * BPER].reshape(2, 4, 2, 32, 128, M)
        xv[:, :, :, :, k1f, :] = flat[:, :, :, valid].transpose(0, 1, 2, 4, 3, 5)
        # nyd: [q, (c, k2), (b2, m)]
        ny = nyq.reshape(4, 2, 32, 2, M).transpose(1, 0, 3, 2, 4)  # c q b2 k2 m
        xv[:, :, :, :, 64, :] = ny
    return out[0], out[1]


def kernel(x: np.ndarray):
    """x: [64, 4096, 128] fp32 -> (re, im) each [64, 4096, 128] fp32."""
    return _run(x, trace=False)


# revision 18
# speedup vs baseline: 2.9786x; 1.1693x over previous
"""Trainium2 Bass kernel: batched FFT along axis 1 of x[64, 4096, 128] (fp32),
returning (real, imag) parts.  8-core data-parallel over the batch axis.

Algorithm (per core, 8 batches): 4-step Cooley-Tukey, N = 128*32,
n = 32*n1 + n2, k = 128*k2 + k1:

    X[128*k2 + k1] = sum_n2 W32[n2,k2] * (W4096[n2*k1] * sum_n1 W128[n1,k1]*x)

Phase 1 (per batch b, m-range mr of 32): stage-1 DFT-128 over n1 with the
  twiddle FOLDED into per-n2 stationaries PQ[n1, (comp, k1=0..63)] -- both
  components packed into one 128-wide output so every PSUM partition carries
  real data.  One f=32 matmul per n2 fills a strided column slot of a PSUM
  tile a[128, (m32, n2-32)] (n2 innermost).
Evict: one ACT copy per PSUM tile -> fp16 B-slab [128=(c,g,jm), (b2,m,n2)].
Transpose: one DVE StreamTranspose per quarter (32x32 blocks) yields
  Bd[(c,g,n2), (b2, m, jm)] fp16 on-chip -- no DRAM bounce.
Phase 2 (per quarter, jp): stage-2 DFT-32 over n2 with wide [64,128]
  conjugate-packing stationaries (direct k1 = 32g+jm and Hermitian
  k1' = 128-q rows in one pass); fp16 moving, free dim (b2, je, m).
  C tiles are dumped verbatim to DRAM; the host relabels (pure data
  movement, no arithmetic).
Nyquist rows (k1 = 64) skip the transpose: X[128k2+64] is computed directly
  from x by a single-stage matmul accumulated over n2 in PSUM.
"""

import numpy as np
from contextlib import ExitStack

import concourse.bacc as bacc
import concourse.bass as bass
import concourse.mybir as mybir
import concourse.tile as tile
from concourse.bass_utils import run_bass_kernel_spmd

N = 4096
N1, N2 = 128, 32
M = 128
B_FULL = 64
NCORES = 8
BPER = B_FULL // NCORES  # 8 batches per core

FP16 = mybir.dt.float16
FP32 = mybir.dt.float32


# ---------------------------------------------------------------- constants
def make_consts():
    n1 = np.arange(N1)
    q = np.arange(64)
    # phase-1 folded stationaries: PQ[n1, n2*128 + c*64 + q], q = k1 = 0..63
    pq = np.zeros((N1, N2 * 128), np.float32)
    for n2 in range(N2):
        theta = 2 * np.pi * np.outer(32 * n1 + n2, q) / N
        pq[:, n2 * 128: n2 * 128 + 64] = np.cos(theta)
        pq[:, n2 * 128 + 64: n2 * 128 + 128] = -np.sin(theta)

    # nyquist single-stage stationaries: NY[n1, n2*64 + c*32 + k2]
    k2 = np.arange(N2)
    nyst = np.zeros((N1, N2 * 64), np.float32)
    for n2 in range(N2):
        th = 2 * np.pi * np.outer(32 * n1 + n2, 128 * k2 + 64) / N
        nyst[:, n2 * 64: n2 * 64 + 32] = np.cos(th)
        nyst[:, n2 * 64 + 32: n2 * 64 + 64] = -np.sin(th)

    # phase-2 wide stationaries [128, 512]: rows 0..63 pair with the
    # re-moving (bd[0:64]), rows 64..127 with the im-moving (bd[64:128]).
    n2v = np.arange(N2)
    a2 = 2 * np.pi * np.outer(n2v, k2) / N2
    a2u = 2 * np.pi * np.outer(n2v, k2 + 1) / N2
    c, s = np.cos(a2), -np.sin(a2)
    cu, su_ = np.cos(a2u), -np.sin(a2u)

    def wide(up, low):
        z = np.zeros((64, 128), np.float32)
        z[0:32, 0:32] = up
        z[32:64, 32:64] = up
        z[32:64, 64:96] = low
        z[0:32, 96:128] = low
        return z

    # stacked [128, 128] stationaries: rows 0..63 act on Bd_re (bd[0:64]),
    # rows 64..127 on Bd_im (bd[64:128]) -- one matmul per C component
    su = np.zeros((128, 256), np.float32)
    su[0:64, 0:128] = wide(c, cu)        # SA: C_re <- Bd_re
    su[64:128, 0:128] = wide(-s, su_)    # SB: C_re <- Bd_im
    su[0:64, 128:256] = wide(s, su_)     # SC: C_im <- Bd_re
    su[64:128, 128:256] = wide(c, -cu)   # SD: C_im <- Bd_im

    return {
        "pq": pq.astype(np.float16),
        "nyst": nyst.astype(np.float16),
        "su": su.astype(np.float16),
    }


def _hand_ap(base_ap, rel_off, dims):
    return bass.AP(tensor=base_ap.tensor, offset=base_ap.offset + rel_off,
                   ap=[list(d) for d in dims])


# ---------------------------------------------------------------- program
def build_program():
    nc = bacc.Bacc("TRN2", target_bir_lowering=False, debug=False)

    x_in = nc.dram_tensor("x", [BPER, N, M], FP16, kind="ExternalInput")
    # raw C-tile dumps: outd[q, jp, po, (c, b2, je, m)]
    outd = nc.dram_tensor("outd", [4, 8, 128, 2048], FP16,
                          kind="ExternalOutput")
    nyd = nc.dram_tensor("nyd", [4, 64, 256], FP16, kind="ExternalOutput")
    pq_in = nc.dram_tensor("pq", [N1, N2 * 128], FP16, kind="ExternalInput")
    nyst_in = nc.dram_tensor("nyst", [N1, N2 * 64], FP16,
                             kind="ExternalInput")
    su_in = nc.dram_tensor("su", [128, 256], FP16, kind="ExternalInput")

    with tile.TileContext(nc) as tc, ExitStack() as ctx:
        cpool = ctx.enter_context(tc.tile_pool(name="consts", bufs=1))
        ct_pq = cpool.tile([N1, N2 * 128], FP16, tag="pq", name="ct_pq")
        ct_ny = cpool.tile([N1, N2 * 64], FP16, tag="nyst", name="ct_ny")
        ct_su = cpool.tile([128, 256], FP16, tag="su", name="ct_su")
        nc.sync.dma_start(ct_pq[:], pq_in.ap())
        nc.sync.dma_start(ct_ny[:], nyst_in.ap())
        nc.sync.dma_start(ct_su[:], su_in.ap())

        x_pool = ctx.enter_context(tc.tile_pool(name="xp", bufs=2))
        a_psum = ctx.enter_context(tc.tile_pool(name="aps", bufs=2, space="PSUM"))
        c_psum = ctx.enter_context(tc.tile_pool(name="cps", bufs=2, space="PSUM"))
        b_pool = ctx.enter_context(tc.tile_pool(name="bp", bufs=2))
        bd_pool = ctx.enter_context(tc.tile_pool(name="bdp", bufs=3))
        cs_pool = ctx.enter_context(tc.tile_pool(name="csp", bufs=3))
        ny_pool = ctx.enter_context(tc.tile_pool(name="nyp", bufs=2))

        def phase1_quarter(qq, bq, nysb):
            """batches 2qq, 2qq+1 -> B-slab [128=(c,q64), (b2, m, n2)] fp16."""
            for eb in range(2):
                b = 2 * qq + eb
                xt = x_pool.tile([N1, N2 * M], FP16, tag="x", name=f"xt_{b}")
                src = _hand_ap(x_in.ap(), b * N * M,
                               [[N2 * M, N1], [1, N2 * M]])
                nc.sync.dma_start(xt[:], src)
                for ng in range(4):   # n2 octets, contiguous psum writes
                    a = a_psum.tile([128, 1024], FP32, tag="a",
                                    name=f"a_{b}_{ng}")
                    for j in range(8):
                        n2 = 8 * ng + j
                        stat = ct_pq[:, n2 * 128:(n2 + 1) * 128]
                        mov = xt[:, n2 * M:(n2 + 1) * M]
                        nc.tensor.matmul(a[:, j * 128:(j + 1) * 128],
                                         stat, mov, start=True, stop=True)
                    # strided eviction performs the (n2, m) -> (m, n2) reorder
                    dst = bq[:].rearrange("p (b m n) -> p b m n",
                                          b=2, m=M, n=N2)[
                        :, eb, :, 8 * ng:8 * ng + 8]
                    srcp = a[:].rearrange("p (n m) -> p m n", n=8, m=M)
                    nc.scalar.copy(dst, srcp)
                # nyquist: single-stage, accumulate over n2 in PSUM
                any_ = a_psum.tile([128, 1024], FP32, tag="a", name=f"any_{b}")
                for n2 in range(N2):
                    stat = ct_ny[:, n2 * 64:(n2 + 1) * 64]
                    mov = xt[:, n2 * M:(n2 + 1) * M]
                    nc.tensor.matmul(any_[0:64, 0:128], stat, mov,
                                     start=(n2 == 0), stop=(n2 == N2 - 1))
                nc.scalar.copy(nysb[:, eb * 128:(eb + 1) * 128],
                               any_[0:64, 0:128])

        def phase2_quarter(qq, bd):
            bd4 = bd[:].rearrange("p (b m j) -> p b j m", b=2, m=M, j=N2)
            for jph in range(8):
                csb = cs_pool.tile([128, 2048], FP16, tag="csb",
                                   name=f"csb_{qq}_{jph}")
                for jpar in range(2):
                    jp = 2 * jph + jpar
                    cc = c_psum.tile([128, 1024], FP32, tag="c",
                                     name=f"c_{qq}_{jp}")
                    mov = bd4[:, :, 2 * jp:2 * jp + 2, :]
                    nc.tensor.matmul(cc[:, 0:512], ct_su[:, 0:128], mov,
                                     start=True, stop=True)
                    nc.tensor.matmul(cc[:, 512:1024], ct_su[:, 128:256], mov,
                                     start=True, stop=True)
                    dstc = csb[:, jpar * 1024:(jpar + 1) * 1024]
                    if jp % 2 == 0:
                        nc.scalar.copy(dstc, cc[:])
                    else:
                        nc.vector.tensor_copy(dstc, cc[:])
                dst = _hand_ap(outd.ap(), (qq * 8 + jph) * 128 * 2048,
                               [[2048, 128], [1, 2048]])
                nc.sync.dma_start(dst, csb[:])

        # ---------------- pipeline ----------------
        for qq in range(4):
            bq = b_pool.tile([128, 2 * M * N2], FP16, tag="b",
                             name=f"bq_{qq}")
            bd = bd_pool.tile([128, 2 * M * N2], FP16, tag="bd",
                              name=f"bd_{qq}")
            nysb = ny_pool.tile([64, 256], FP16, tag="nsb", name=f"nysb_{qq}")
            phase1_quarter(qq, bq, nysb)
            nc.vector.transpose(bd[:], bq[:])
            dstn = _hand_ap(nyd.ap(), qq * 64 * 256, [[256, 64], [1, 256]])
            nc.sync.dma_start(dstn, nysb[:])
            phase2_quarter(qq, bd)

    nc.compile()
    return nc


_CACHE = {}


def _get_program():
    if "nc" not in _CACHE:
        _CACHE["nc"] = build_program()
        _CACHE["consts"] = make_consts()
    return _CACHE["nc"], _CACHE["consts"]


_LAST = {}


def _k1_map():
    k1 = np.zeros((16, 4, 2), np.int64)
    jp = np.arange(16)[:, None]
    je = np.arange(2)[None, :]
    k1[:, 0, :] = 2 * jp + je
    k1[:, 1, :] = 32 + 2 * jp + je
    k1[:, 2, :] = 96 - 2 * jp - je
    k1[:, 3, :] = 128 - 2 * jp - je
    valid = (k1 < 128).reshape(-1)   # drop jp=0,G3,je=0 (k1=128 wrap dup)
    return k1, valid


def _run(x: np.ndarray, trace: bool = False):
    x = np.asarray(x)
    assert x.shape == (B_FULL, N, M)
    x16 = np.ascontiguousarray(x.astype(np.float16))
    nc, consts = _get_program()
    in_maps = []
    for c in range(NCORES):
        m = {"x": np.ascontiguousarray(x16[c * BPER:(c + 1) * BPER])}
        m.update(consts)
        in_maps.append(m)
    bres = run_bass_kernel_spmd(nc, in_maps, list(range(NCORES)), trace=trace)
    _LAST["results"] = bres
    res = bres.results
    out = np.empty((2, B_FULL, N, M), np.float32)
    k1m, valid = _k1_map()
    k1f = k1m.reshape(-1)[valid]
    for core in range(NCORES):
        scr = res[core]["outd"].astype(np.float32)
        nyq = res[core]["nyd"].astype(np.float32)
        # scr: [q, jph, po, f=(jpar c b2 je m)] -> (c q b2 jph jpar G je k2 m)
        s = scr.reshape(4, 8, 4, 32, 2, 2, 2, 2, M)
        p = s.transpose(5, 0, 6, 1, 4, 2, 7, 3, 8)
        flat = p.reshape(2, 4, 2, 16 * 4 * 2, 32, M)
        xv = out[:, core * BPER:(core + 1) * BPER].reshape(2, 4, 2, 32, 128, M)
        xv[:, :, :, :, k1f, :] = flat[:, :, :, valid].transpose(0, 1, 2, 4, 3, 5)
        # nyd: [q, (c, k2), (b2, m)]
        ny = nyq.reshape(4, 2, 32, 2, M).transpose(1, 0, 3, 2, 4)  # c q b2 k2 m
        xv[:, :, :, :, 64, :] = ny
    return out[0], out[1]


def kernel(x: np.ndarray):
    """x: [64, 4096, 128] fp32 -> (re, im) each [64, 4096, 128] fp32."""
    return _run(x, trace=False)


# revision 22
# speedup vs baseline: 3.6357x; 1.2206x over previous
"""Trainium2 Bass kernel: batched FFT along axis 1 of x[64, 4096, 128] (fp32),
returning (real, imag) parts.  8-core data-parallel over the batch axis.

Algorithm (per core, 8 batches): 4-step Cooley-Tukey, N = 128*32,
n = 32*n1 + n2, k = 128*k2 + k1:

    X[128*k2 + k1] = sum_n2 W32[n2,k2] * (W4096[n2*k1] * sum_n1 W128[n1,k1]*x)

Phase 1 (per batch b, m-range mr of 32): stage-1 DFT-128 over n1 with the
  twiddle FOLDED into per-n2 stationaries PQ[n1, (comp, k1=0..63)] -- both
  components packed into one 128-wide output so every PSUM partition carries
  real data.  One f=32 matmul per n2 fills a strided column slot of a PSUM
  tile a[128, (m32, n2-32)] (n2 innermost).
Evict: one ACT copy per PSUM tile -> fp16 B-slab [128=(c,g,jm), (b2,m,n2)].
Transpose: one DVE StreamTranspose per quarter (32x32 blocks) yields
  Bd[(c,g,n2), (b2, m, jm)] fp16 on-chip -- no DRAM bounce.
Phase 2 (per quarter, jp): stage-2 DFT-32 over n2 with wide [64,128]
  conjugate-packing stationaries (direct k1 = 32g+jm and Hermitian
  k1' = 128-q rows in one pass); fp16 moving, free dim (b2, je, m).
  C tiles are dumped verbatim to DRAM; the host relabels (pure data
  movement, no arithmetic).
Nyquist rows (k1 = 64) skip the transpose: X[128k2+64] is computed directly
  from x by a single-stage matmul accumulated over n2 in PSUM.
"""

import numpy as np
from contextlib import ExitStack

import concourse.bacc as bacc
import concourse.bass as bass
import concourse.mybir as mybir
import concourse.tile as tile
from concourse.bass_utils import run_bass_kernel_spmd

N = 4096
N1, N2 = 128, 32
M = 128
B_FULL = 64
NCORES = 8
BPER = B_FULL // NCORES  # 8 batches per core

FP16 = mybir.dt.float16
FP32 = mybir.dt.float32


# ---------------------------------------------------------------- constants
def make_consts():
    n1 = np.arange(N1)
    q = np.arange(64)
    # phase-1 folded stationaries: PQ[n1, n2*128 + c*64 + q], q = k1 = 0..63
    pq = np.zeros((N1, N2 * 128), np.float32)
    for n2 in range(N2):
        theta = 2 * np.pi * np.outer(32 * n1 + n2, q) / N
        pq[:, n2 * 128: n2 * 128 + 64] = np.cos(theta)
        pq[:, n2 * 128 + 64: n2 * 128 + 128] = -np.sin(theta)

    # nyquist single-stage stationaries: NY[n1, n2*64 + c*32 + k2]
    k2 = np.arange(N2)
    nyst = np.zeros((N1, N2 * 64), np.float32)
    for n2 in range(N2):
        th = 2 * np.pi * np.outer(32 * n1 + n2, 128 * k2 + 64) / N
        nyst[:, n2 * 64: n2 * 64 + 32] = np.cos(th)
        nyst[:, n2 * 64 + 32: n2 * 64 + 64] = -np.sin(th)

    # phase-2 stationaries, direct rows only (k1 = 32g+jm, G in {0,1}); the
    # Hermitian half (k1 = 65..127) is mirrored on the host from these.
    n2v = np.arange(N2)
    a2 = 2 * np.pi * np.outer(n2v, k2) / N2
    c, s = np.cos(a2), -np.sin(a2)

    def bdiag(up):
        z = np.zeros((64, 64), np.float32)
        z[0:32, 0:32] = up
        z[32:64, 32:64] = up
        return z

    # stacked [128, 64] stationaries: rows 0..63 act on Bd_re (bd[0:64]),
    # rows 64..127 on Bd_im (bd[64:128]) -- one matmul per C component
    su = np.zeros((128, 128), np.float32)
    su[0:64, 0:64] = bdiag(c)       # C_re <- Bd_re
    su[64:128, 0:64] = bdiag(-s)    # C_re <- Bd_im
    su[0:64, 64:128] = bdiag(s)     # C_im <- Bd_re
    su[64:128, 64:128] = bdiag(c)   # C_im <- Bd_im

    return {
        "pq": pq.astype(np.float16),
        "nyst": nyst.astype(np.float16),
        "su": su.astype(np.float16),
    }


def _hand_ap(base_ap, rel_off, dims):
    return bass.AP(tensor=base_ap.tensor, offset=base_ap.offset + rel_off,
                   ap=[list(d) for d in dims])


# ---------------------------------------------------------------- program
def build_program():
    nc = bacc.Bacc("TRN2", target_bir_lowering=False, debug=False)

    x_in = nc.dram_tensor("x", [BPER, N, M], FP16, kind="ExternalInput")
    # raw C-tile dumps: outd[q, jp, po, (c, b2, je, m)]
    outd = nc.dram_tensor("outd", [4, 4, 128, 2048], FP16,
                          kind="ExternalOutput")
    nyd = nc.dram_tensor("nyd", [4, 64, 256], FP16, kind="ExternalOutput")
    pq_in = nc.dram_tensor("pq", [N1, N2 * 128], FP16, kind="ExternalInput")
    nyst_in = nc.dram_tensor("nyst", [N1, N2 * 64], FP16,
                             kind="ExternalInput")
    su_in = nc.dram_tensor("su", [128, 128], FP16, kind="ExternalInput")

    with tile.TileContext(nc) as tc, ExitStack() as ctx:
        cpool = ctx.enter_context(tc.tile_pool(name="consts", bufs=1))
        ct_pq = cpool.tile([N1, N2 * 128], FP16, tag="pq", name="ct_pq")
        ct_ny = cpool.tile([N1, N2 * 64], FP16, tag="nyst", name="ct_ny")
        ct_su = cpool.tile([128, 128], FP16, tag="su", name="ct_su")
        nc.sync.dma_start(ct_pq[:], pq_in.ap())
        nc.sync.dma_start(ct_ny[:], nyst_in.ap())
        nc.sync.dma_start(ct_su[:], su_in.ap())

        x_pool = ctx.enter_context(tc.tile_pool(name="xp", bufs=2))
        a_psum = ctx.enter_context(tc.tile_pool(name="aps", bufs=2, space="PSUM"))
        c_psum = ctx.enter_context(tc.tile_pool(name="cps", bufs=2, space="PSUM"))
        b_pool = ctx.enter_context(tc.tile_pool(name="bp", bufs=2))
        bd_pool = ctx.enter_context(tc.tile_pool(name="bdp", bufs=3))
        cs_pool = ctx.enter_context(tc.tile_pool(name="csp", bufs=3))
        ny_pool = ctx.enter_context(tc.tile_pool(name="nyp", bufs=2))

        def phase1_quarter(qq, bq, nysb):
            """batches 2qq, 2qq+1 -> B-slab [128=(c,q64), (b2, m, n2)] fp16."""
            for eb in range(2):
                b = 2 * qq + eb
                xt = x_pool.tile([N1, N2 * M], FP16, tag="x", name=f"xt_{b}")
                src = _hand_ap(x_in.ap(), b * N * M,
                               [[N2 * M, N1], [1, N2 * M]])
                nc.sync.dma_start(xt[:], src)
                for ng in range(4):   # n2 octets, contiguous psum writes
                    a = a_psum.tile([128, 1024], FP32, tag="a",
                                    name=f"a_{b}_{ng}")
                    for j in range(8):
                        n2 = 8 * ng + j
                        stat = ct_pq[:, n2 * 128:(n2 + 1) * 128]
                        mov = xt[:, n2 * M:(n2 + 1) * M]
                        nc.tensor.matmul(a[:, j * 128:(j + 1) * 128],
                                         stat, mov, start=True, stop=True)
                    # strided eviction performs the (n2, m) -> (m, n2) reorder
                    dst = bq[:].rearrange("p (b m n) -> p b m n",
                                          b=2, m=M, n=N2)[
                        :, eb, :, 8 * ng:8 * ng + 8]
                    srcp = a[:].rearrange("p (n m) -> p m n", n=8, m=M)
                    nc.scalar.copy(dst, srcp)
                # nyquist: single-stage, accumulate over n2 in PSUM
                any_ = a_psum.tile([128, 1024], FP32, tag="a", name=f"any_{b}")
                for n2 in range(N2):
                    stat = ct_ny[:, n2 * 64:(n2 + 1) * 64]
                    mov = xt[:, n2 * M:(n2 + 1) * M]
                    nc.tensor.matmul(any_[0:64, 0:128], stat, mov,
                                     start=(n2 == 0), stop=(n2 == N2 - 1))
                nc.scalar.copy(nysb[:, eb * 128:(eb + 1) * 128],
                               any_[0:64, 0:128])

        def phase2_quarter(qq, bd):
            bd4 = bd[:].rearrange("p (b m j) -> p b j m", b=2, m=M, j=N2)
            for dh in range(4):
                csb = cs_pool.tile([128, 2048], FP16, tag="csb",
                                   name=f"csb_{qq}_{dh}")
                for pp in range(2):
                    cc = c_psum.tile([128, 1024], FP32, tag="c",
                                     name=f"c_{qq}_{dh}_{pp}")
                    for jpar in range(2):
                        jp = 4 * dh + 2 * pp + jpar
                        mov = bd4[:, :, 2 * jp:2 * jp + 2, :]
                        ps = slice(64 * jpar, 64 * jpar + 64)
                        nc.tensor.matmul(cc[ps, 0:512], ct_su[:, 0:64],
                                         mov, start=True, stop=True)
                        nc.tensor.matmul(cc[ps, 512:1024], ct_su[:, 64:128],
                                         mov, start=True, stop=True)
                    dstc = csb[:, pp * 1024:(pp + 1) * 1024]
                    if pp == 0:
                        nc.scalar.copy(dstc, cc[:])
                    else:
                        nc.vector.tensor_copy(dstc, cc[:])
                dst = _hand_ap(outd.ap(), (qq * 4 + dh) * 128 * 2048,
                               [[2048, 128], [1, 2048]])
                nc.sync.dma_start(dst, csb[:])

        # ---------------- pipeline ----------------
        for qq in range(4):
            bq = b_pool.tile([128, 2 * M * N2], FP16, tag="b",
                             name=f"bq_{qq}")
            bd = bd_pool.tile([128, 2 * M * N2], FP16, tag="bd",
                              name=f"bd_{qq}")
            nysb = ny_pool.tile([64, 256], FP16, tag="nsb", name=f"nysb_{qq}")
            phase1_quarter(qq, bq, nysb)
            nc.vector.transpose(bd[:], bq[:])
            dstn = _hand_ap(nyd.ap(), qq * 64 * 256, [[256, 64], [1, 256]])
            nc.sync.dma_start(dstn, nysb[:])
            phase2_quarter(qq, bd)

    nc.compile()
    return nc


_CACHE = {}


def _get_program():
    if "nc" not in _CACHE:
        _CACHE["nc"] = build_program()
        _CACHE["consts"] = make_consts()
    return _CACHE["nc"], _CACHE["consts"]


_LAST = {}


def _k1_map():
    k1 = np.zeros((16, 2, 2), np.int64)
    jp = np.arange(16)[:, None]
    je = np.arange(2)[None, :]
    k1[:, 0, :] = 2 * jp + je
    k1[:, 1, :] = 32 + 2 * jp + je
    return k1


def _run(x: np.ndarray, trace: bool = False):
    x = np.asarray(x)
    assert x.shape == (B_FULL, N, M)
    x16 = np.ascontiguousarray(x.astype(np.float16))
    nc, consts = _get_program()
    in_maps = []
    for c in range(NCORES):
        m = {"x": np.ascontiguousarray(x16[c * BPER:(c + 1) * BPER])}
        m.update(consts)
        in_maps.append(m)
    bres = run_bass_kernel_spmd(nc, in_maps, list(range(NCORES)), trace=trace)
    _LAST["results"] = bres
    res = bres.results
    out = np.empty((2, B_FULL, N, M), np.float32)
    k1f = _k1_map().reshape(-1)
    for core in range(NCORES):
        scr = res[core]["outd"].astype(np.float32)
        nyq = res[core]["nyd"].astype(np.float32)
        # scr: [q, dh, po=(jpar G k2), f=(pp c b2 je m)]
        #   -> (c q b2 dh pp jpar G je k2 m); jp = 4dh + 2pp + jpar
        s = scr.reshape(4, 4, 2, 2, 32, 2, 2, 2, 2, M)
        p = s.transpose(6, 0, 7, 1, 5, 2, 3, 8, 4, 9)
        flat = p.reshape(2, 4, 2, 16 * 2 * 2, 32, M)
        xv = out[:, core * BPER:(core + 1) * BPER].reshape(2, 4, 2, 32, 128, M)
        xv[:, :, :, :, k1f, :] = flat.transpose(0, 1, 2, 4, 3, 5)
        # nyd: [q, (c, k2), (b2, m)]
        ny = nyq.reshape(4, 2, 32, 2, M).transpose(1, 0, 3, 2, 4)  # c q b2 k2 m
        xv[:, :, :, :, 64, :] = ny
    # Hermitian mirror: X[N-k] = conj(X[k]) (k1' = 65..127 from k1 = 63..1,
    # k2' = 31-k2); axes of full[c] are (b, k2, k1, m)
    full = out.reshape(2, B_FULL, 32, 128, M)
    full[0][:, :, 65:, :] = full[0][:, ::-1, 63:0:-1, :]
    full[1][:, :, 65:, :] = -full[1][:, ::-1, 63:0:-1, :]
    return out[0], out[1]


def kernel(x: np.ndarray):
    """x: [64, 4096, 128] fp32 -> (re, im) each [64, 4096, 128] fp32."""
    return _run(x, trace=False)
